# revision 1
# baseline (speedup 1.0000x reference)
"""Trainium2 Bass kernel for nn_BackEdgeConv2d (threshold -> reflect-pad 7x7
box-count -> tolerance-band mask -> zero masked pixels).

Self-contained: hardcodes shapes [16, 3, 1024, 1024] f32 and the 8-core
batch-parallel sharding (2 images = 6 HxW planes per core).

Math (exact, no approximation):
  cond = (x >= 128/255)                            in {0,1}
  csum = reflect-pad 7x7 box sum of cond           in [0, 49]
  mask = 4.8 <= csum <= 19.2  <=>  5 <= csum <= 19
  out  = x * (1 - mask)

Implemented in a signed domain s = 2*cond - 1 = Sign(x - t + eps) so the
threshold is a single ScalarE activation; then S = boxsum(s) = 2*csum - 49
and mask <=> |S + 25| <= 14. All intermediates are exactly representable
(bf16 holds small integers exactly; PSUM accumulates in fp32).

Per 128-row tile pipeline (default _emit_v5, chosen by engine-isolation
profiling: DVE was the bottleneck at ~112us vs PE ~24us):
  1MiB paired DMA in -> ACT Sign (thresh, +-1 bf16; reflect W-pad via
  reversed-AP ACT copies) -> DVE 3 shifted bf16 adds (6-tap W sum s3)
  -> PE: 12 matmuls/tile accumulating in PSUM both the H-direction band
  conv of s3 AND a second chain on raw signs shifted by 6 (the 7th W tap,
  by linearity) -> ACT Abs(S+25) from PSUM -> DVE fused (|.|>15)*x
  -> 1MiB paired DMA out. All values exact; ~130-145us/core measured,
  vs engine floors DVE ~90, DMA ~55-80, PE ~48, ACT ~43.
"""

import contextlib
import os

os.environ.setdefault("MYCRO_LOCAL_CACHE", "1")

import numpy as np
import ml_dtypes

import concourse.bass as bass
import concourse.mybir as mybir
import concourse.tile as tile
from concourse.bacc import Bacc
from concourse.bass_utils import run_bass_kernel_spmd

F32 = mybir.dt.float32
BF16 = mybir.dt.bfloat16

B, C, H, W = 16, 3, 1024, 1024
N_CORES = 8
IMGS_PER_CORE = B // N_CORES          # 2
PLANES = IMGS_PER_CORE * C            # 6 HxW planes per core
PT = 128                              # partition tile height
KS, PAD = 7, 3
CHUNK = 512                           # psum bank free-dim size (fp32)

# fp32 threshold and the epsilon-shifted sign bias:
#   x >= t  <=>  x - (t - 2^-24) > 0   for x a multiple of 2^-23 (jax uniform)
_T = np.float32(128.0 / 255.0)
SIGN_BIAS = -float(np.float32(float(_T) - 2.0 ** -24))
# v9 threshold: x - T is always an ODD multiple of 2^-24 (T_f32 has odd
# mantissa, x is on the 2^-23 grid), so x >= T <=> x - T > 0 with NO epsilon.
# Pre-scaling by 2^20 keeps |input| >= 1/16 so the ACT Sign table never sees
# a tiny value it would map to 0 (observed hw behavior at |z| ~ 2^-24).
SIGN_SCALE = float(2.0 ** 20)
SIGN_BIAS_SC = -float(np.float32(_T) * np.float32(SIGN_SCALE))

# band-matrix indices in the packed "bands" input
BP, BM, BN, BT, BB = 0, 1, 2, 3, 4
# fp8 band-PAIR indices in the packed "bands8" input (for DoubleRow chains)
PR_PM, PR_ZN, PR_TN, PR_PB = 0, 1, 2, 3
PR_PP, PR_MM, PR_NN, PR_TT, PR_B2 = 4, 5, 6, 7, 8
F8 = mybir.dt.float8e4
DRMODE = mybir.MatmulPerfMode.DoubleRow


def _band_blocks(h: int) -> np.ndarray:
    """5 x [128,128] H-direction band matrices (prev/mid/next/top/bottom)
    for a reflect-padded 7-tap column sum, sliced from the full h x h
    convolution matrix. M[r_in, r_out] = multiplicity of row r_in in the
    7-tap reflect window of output row r_out."""
    m = np.zeros((h, h), np.float32)
    for j in range(h):
        for d in range(-PAD, PAD + 1):
            r = j + d
            if r < 0:
                r = -r
            elif r >= h:
                r = 2 * (h - 1) - r
            m[r, j] += 1.0
    assert h >= 3 * PT
    blocks = np.stack([
        m[0:PT, PT:2 * PT],            # BP: tile t-1 rows -> out tile t
        m[PT:2 * PT, PT:2 * PT],       # BM: tile t rows -> out tile t
        m[2 * PT:3 * PT, PT:2 * PT],   # BN: tile t+1 rows -> out tile t
        m[0:PT, 0:PT],                 # BT: top tile (reflect folded)
        m[h - PT:h, h - PT:h],         # BB: bottom tile (reflect folded)
    ])
    return blocks.astype(ml_dtypes.bfloat16)


def _emit(nc, x_d, bands_d, out_d, planes: int, h: int, w: int,
          reps: int = 1) -> None:
    """Emit the full per-core kernel body (opens its own TileContext).

    reps > 1 repeats the whole pass back-to-back inside one NEFF; used only
    for timing (amplifies kernel time above the dispatch overhead)."""
    nt = h // PT
    assert h % PT == 0 and nt >= 2 and w % CHUNK == 0
    nchunks = w // CHUNK

    AF = mybir.ActivationFunctionType
    OP = mybir.AluOpType

    with tile.TileContext(nc) as tc:
        with (
            tc.tile_pool(name="consts", bufs=1) as cp,
            tc.tile_pool(name="xin", bufs=5) as xp,
            tc.tile_pool(name="thr", bufs=3) as thp,
            tc.tile_pool(name="wsum", bufs=3) as wp,
            tc.tile_pool(name="s7p", bufs=5) as s7p,
            tc.tile_pool(name="absp", bufs=3) as ap_pool,
            tc.tile_pool(name="outp", bufs=3) as op_pool,
            tc.tile_pool(name="psum", bufs=4, space="PSUM") as psp,
        ):
            bands_sb = cp.tile([PT, 5, PT], BF16)
            nc.sync.dma_start(bands_sb[:], bands_d.rearrange("m i j -> i m j"))
            bias_thr = cp.tile([PT, 1], F32)
            nc.gpsimd.memset(bias_thr[:], SIGN_BIAS)
            bias_25 = cp.tile([PT, 1], F32)
            nc.gpsimd.memset(bias_25[:], 25.0)

            for p in [pp for _ in range(reps) for pp in range(planes)]:
                x_ring: dict[int, bass.AP] = {}
                s7_ring: dict[int, bass.AP] = {}
                for t in range(nt + 1):
                    if t < nt:
                        # load 128 rows, threshold to signs, 7-tap W-sum
                        xt = xp.tile([PT, w], F32, tag="x")
                        nc.sync.dma_start(xt[:], x_d[p, t * PT:(t + 1) * PT, :])
                        x_ring[t] = xt

                        ce = thp.tile([PT, w + 6], BF16, tag="ce")
                        nc.scalar.activation(ce[:, 3:w + 3], xt[:], AF.Sign,
                                             bias=bias_thr[:])
                        # reflect pad in W (cols 0..2 and w+3..w+5)
                        nc.vector.tensor_copy(ce[:, 0:3], ce[:, 6:3:-1])
                        nc.vector.tensor_copy(ce[:, w + 3:w + 6],
                                              ce[:, w + 1:w - 2:-1])

                        s1 = wp.tile([PT, w + 4], BF16, tag="s1")
                        nc.vector.tensor_tensor(s1[:], ce[:, 0:w + 4],
                                                ce[:, 1:w + 5], OP.add)
                        s2 = wp.tile([PT, w], BF16, tag="s2")
                        nc.vector.tensor_tensor(s2[:], s1[:, 0:w],
                                                s1[:, 2:w + 2], OP.add)
                        s3 = wp.tile([PT, w], BF16, tag="s3")
                        nc.vector.tensor_tensor(s3[:], s2[:], s1[:, 4:w + 4],
                                                OP.add)
                        s7 = s7p.tile([PT, w], BF16, tag="s7")
                        nc.vector.tensor_tensor(s7[:], s3[:], ce[:, 6:w + 6],
                                                OP.add)
                        s7_ring[t] = s7

                    u = t - 1
                    if u < 0:
                        continue
                    # H-direction band matmuls + mask + blend for out tile u
                    if u == 0:
                        mms = [(BT, s7_ring[0]), (BN, s7_ring[1])]
                    elif u == nt - 1:
                        mms = [(BP, s7_ring[u - 1]), (BB, s7_ring[u])]
                    else:
                        mms = [(BP, s7_ring[u - 1]), (BM, s7_ring[u]),
                               (BN, s7_ring[u + 1])]

                    a = ap_pool.tile([PT, w], BF16, tag="a")
                    for c in range(nchunks):
                        sl = slice(c * CHUNK, (c + 1) * CHUNK)
                        ps = psp.tile([PT, CHUNK], F32, tag="ps")
                        for k, (mi, s7src) in enumerate(mms):
                            nc.tensor.matmul(ps[:], bands_sb[:, mi, :],
                                             s7src[:, sl],
                                             start=(k == 0),
                                             stop=(k == len(mms) - 1))
                        # a = |S + 25|; mask <=> a <= 14 (a is an even int)
                        nc.scalar.activation(a[:, sl], ps[:], AF.Abs,
                                             bias=bias_25[:])
                    ot = op_pool.tile([PT, w], F32, tag="ot")
                    # out = (a > 15) * x  : keep pixel iff out of band
                    nc.vector.scalar_tensor_tensor(ot[:], a[:], 15.0,
                                                   x_ring[u][:],
                                                   OP.is_gt, OP.mult)
                    nc.sync.dma_start(out_d[p, u * PT:(u + 1) * PT, :], ot[:])


def _emit_v2(nc, x_d, bands_d, out_d, planes: int, h: int, w: int,
             reps: int = 1) -> None:
    """Optimized emit: 1 MiB paired DMAs (2 row-tiles per transfer), one
    2-bank PSUM tile + single Abs per out tile, weight-grouped matmuls."""
    nt = h // PT
    assert h % PT == 0 and nt >= 2 and w % CHUNK == 0
    nchunks = w // CHUNK

    AF = mybir.ActivationFunctionType
    OP = mybir.AluOpType

    with tile.TileContext(nc) as tc:
        with (
            tc.tile_pool(name="consts", bufs=1) as cp,
            tc.tile_pool(name="xin", bufs=4) as xp,
            tc.tile_pool(name="thr", bufs=3) as thp,
            tc.tile_pool(name="wsum", bufs=3) as wp,
            tc.tile_pool(name="s7p", bufs=5) as s7p,
            tc.tile_pool(name="absp", bufs=3) as ap_pool,
            tc.tile_pool(name="outp", bufs=3) as op_pool,
            tc.tile_pool(name="psum", bufs=3, space="PSUM") as psp,
        ):
            bands_sb = cp.tile([PT, 5, PT], BF16)
            nc.sync.dma_start(bands_sb[:], bands_d.rearrange("m i j -> i m j"))
            bias_thr = cp.tile([PT, 1], F32)
            nc.gpsimd.memset(bias_thr[:], SIGN_BIAS)
            bias_25 = cp.tile([PT, 1], F32)
            nc.gpsimd.memset(bias_25[:], 25.0)

            for p in [pp for _ in range(reps) for pp in range(planes)]:
                x_ring: dict[int, bass.AP] = {}
                s7_ring: dict[int, bass.AP] = {}
                ot_group: dict[int, bass.AP] = {}
                for t in range(nt + 1):
                    if t < nt:
                        if t % 2 == 0:
                            # load 2 row-tiles (1 MiB) in one DMA when possible
                            gsz = 2 if t + 1 < nt else 1
                            xt = xp.tile([PT, 2, w], F32, tag="x")
                            src = x_d[p, t * PT:(t + gsz) * PT, :]
                            nc.sync.dma_start(
                                xt[:, 0:gsz, :],
                                src.rearrange("(c q) w -> q c w", q=PT))
                            x_ring[t] = xt[:, 0, :]
                            if gsz == 2:
                                x_ring[t + 1] = xt[:, 1, :]
                        xv = x_ring[t]

                        ce = thp.tile([PT, w + 6], BF16, tag="ce")
                        nc.scalar.activation(ce[:, 3:w + 3], xv, AF.Sign,
                                             bias=bias_thr[:])
                        # reflect pad in W on ACT (keeps DVE for the adds)
                        nc.scalar.activation(ce[:, 0:3], ce[:, 6:3:-1],
                                             AF.Copy, bias=0.0)
                        nc.scalar.activation(ce[:, w + 3:w + 6],
                                             ce[:, w + 1:w - 2:-1],
                                             AF.Copy, bias=0.0)

                        s1 = wp.tile([PT, w + 4], BF16, tag="s1")
                        nc.vector.tensor_tensor(s1[:], ce[:, 0:w + 4],
                                                ce[:, 1:w + 5], OP.add)
                        s2 = wp.tile([PT, w], BF16, tag="s2")
                        nc.vector.tensor_tensor(s2[:], s1[:, 0:w],
                                                s1[:, 2:w + 2], OP.add)
                        s3 = wp.tile([PT, w], BF16, tag="s3")
                        nc.vector.tensor_tensor(s3[:], s2[:], s1[:, 4:w + 4],
                                                OP.add)
                        s7 = s7p.tile([PT, w], BF16, tag="s7")
                        nc.vector.tensor_tensor(s7[:], s3[:], ce[:, 6:w + 6],
                                                OP.add)
                        s7_ring[t] = s7

                    u = t - 1
                    if u < 0:
                        continue
                    if u == 0:
                        mms = [(BT, s7_ring[0]), (BN, s7_ring[1])]
                    elif u == nt - 1:
                        mms = [(BP, s7_ring[u - 1]), (BB, s7_ring[u])]
                    else:
                        mms = [(BP, s7_ring[u - 1]), (BM, s7_ring[u]),
                               (BN, s7_ring[u + 1])]

                    # 2-bank psum tile; weight-grouped order (chunk inner)
                    ps = psp.tile([PT, nchunks, CHUNK], F32, tag="ps")
                    for k, (mi, s7src) in enumerate(mms):
                        for c in range(nchunks):
                            nc.tensor.matmul(
                                ps[:, c, :], bands_sb[:, mi, :],
                                s7src[:, c * CHUNK:(c + 1) * CHUNK],
                                start=(k == 0),
                                stop=(k == len(mms) - 1))
                    a = ap_pool.tile([PT, w], BF16, tag="a")
                    nc.scalar.activation(a[:], ps.rearrange("q c k -> q (c k)"),
                                         AF.Abs, bias=bias_25[:])

                    if u % 2 == 0:
                        gsz = 2 if u + 1 < nt else 1
                        ot = op_pool.tile([PT, 2, w], F32, tag="ot")
                        ot_group[u] = ot
                    else:
                        ot = ot_group[u - 1]
                        gsz = 2
                    nc.vector.scalar_tensor_tensor(ot[:, u % 2, :], a[:], 15.0,
                                                   x_ring[u], OP.is_gt, OP.mult)
                    if u % 2 == 1 or u == nt - 1:
                        u0 = u - (u % 2)
                        g = u - u0 + 1
                        dst = out_d[p, u0 * PT:(u0 + g) * PT, :]
                        nc.sync.dma_start(
                            dst.rearrange("(c q) w -> q c w", q=PT),
                            ot[:, 0:g, :])


def _emit_v6(nc, x_d, bands_d, out_d, planes, h, w, reps=1):
    """2 DVE W-adds; psum = sum_nb B@s2 + B@shift4(s1) + B@shift6(raw):
    the last two box taps are folded into the PE accumulation as extra
    shifted-AP matmul chains (18 matmuls/tile). DVE does only s1, s2 and
    the fused compare-multiply blend."""
    nt = h // PT
    assert h % PT == 0 and nt >= 2 and w % CHUNK == 0
    nchunks = w // CHUNK

    AF = mybir.ActivationFunctionType
    OP = mybir.AluOpType

    with tile.TileContext(nc) as tc:
        with (
            tc.tile_pool(name="consts", bufs=1) as cp,
            tc.tile_pool(name="xin", bufs=4) as xp,
            tc.tile_pool(name="thr", bufs=5) as thp,
            tc.tile_pool(name="s1p", bufs=5) as s1p,
            tc.tile_pool(name="s2p", bufs=5) as s2p,
            tc.tile_pool(name="absp", bufs=3) as ap_pool,
            tc.tile_pool(name="outp", bufs=3) as op_pool,
            tc.tile_pool(name="psum", bufs=3, space="PSUM") as psp,
        ):
            bands_sb = cp.tile([PT, 5, PT], BF16)
            nc.sync.dma_start(bands_sb[:], bands_d.rearrange("m i j -> i m j"))
            bias_thr = cp.tile([PT, 1], F32)
            nc.gpsimd.memset(bias_thr[:], SIGN_BIAS)
            bias_25 = cp.tile([PT, 1], F32)
            nc.gpsimd.memset(bias_25[:], 25.0)

            for p in [pp for _ in range(reps) for pp in range(planes)]:
                x_ring: dict[int, bass.AP] = {}
                ce_ring: dict[int, bass.AP] = {}
                s1_ring: dict[int, bass.AP] = {}
                s2_ring: dict[int, bass.AP] = {}
                ot_group: dict[int, bass.AP] = {}
                for t in range(nt + 1):
                    if t < nt:
                        if t % 2 == 0:
                            gsz = 2 if t + 1 < nt else 1
                            xt = xp.tile([PT, 2, w], F32, tag="x")
                            src = x_d[p, t * PT:(t + gsz) * PT, :]
                            nc.sync.dma_start(
                                xt[:, 0:gsz, :],
                                src.rearrange("(c q) w -> q c w", q=PT))
                            x_ring[t] = xt[:, 0, :]
                            if gsz == 2:
                                x_ring[t + 1] = xt[:, 1, :]
                        xv = x_ring[t]

                        # ce holds signs with reflect pad (3 each side)
                        ce = thp.tile([PT, w + 6], BF16, tag="ce")
                        nc.scalar.activation(ce[:, 3:w + 3], xv, AF.Sign,
                                             bias=bias_thr[:])
                        nc.scalar.activation(ce[:, 0:3], ce[:, 6:3:-1],
                                             AF.Copy, bias=0.0)
                        nc.scalar.activation(ce[:, w + 3:w + 6],
                                             ce[:, w + 1:w - 2:-1],
                                             AF.Copy, bias=0.0)
                        ce_ring[t] = ce

                        # W partial sums: s1 pairs, s2 quads (2 bf16 adds)
                        s1 = s1p.tile([PT, w + 4], BF16, tag="s1")
                        nc.vector.tensor_tensor(s1[:], ce[:, 0:w + 4],
                                                ce[:, 1:w + 5], OP.add)
                        s2 = s2p.tile([PT, w], BF16, tag="s2")
                        nc.vector.tensor_tensor(s2[:], s1[:, 0:w],
                                                s1[:, 2:w + 2], OP.add)
                        s1_ring[t] = s1
                        s2_ring[t] = s2

                    u = t - 1
                    if u < 0:
                        continue
                    if u == 0:
                        mms = [(BT, 0), (BN, 1)]
                    elif u == nt - 1:
                        mms = [(BP, u - 1), (BB, u)]
                    else:
                        mms = [(BP, u - 1), (BM, u), (BN, u + 1)]

                    ps = psp.tile([PT, nchunks, CHUNK], F32, tag="ps")
                    chains = (
                        [(s2_ring[st], 0) for _, st in mms]
                        + [(s1_ring[st], 4) for _, st in mms]
                        + [(ce_ring[st], 6) for _, st in mms]
                    )
                    lhs = [bands_sb[:, mi, :] for mi, _ in mms] * 3
                    nmm = len(chains)
                    for k, ((srct, off), lh) in enumerate(zip(chains, lhs)):
                        for c in range(nchunks):
                            nc.tensor.matmul(
                                ps[:, c, :], lh,
                                srct[:, c * CHUNK + off:c * CHUNK + off + CHUNK],
                                start=(k == 0), stop=(k == nmm - 1))
                    a = ap_pool.tile([PT, w], BF16, tag="a")
                    nc.scalar.activation(a[:], ps.rearrange("q c k -> q (c k)"),
                                         AF.Abs, bias=bias_25[:])

                    if u % 2 == 0:
                        ot = op_pool.tile([PT, 2, w], F32, tag="ot")
                        ot_group[u] = ot
                    else:
                        ot = ot_group[u - 1]
                    nc.vector.scalar_tensor_tensor(ot[:, u % 2, :], a[:], 15.0,
                                                   x_ring[u], OP.is_gt, OP.mult)
                    if u % 2 == 1 or u == nt - 1:
                        u0 = u - (u % 2)
                        g = u - u0 + 1
                        dst = out_d[p, u0 * PT:(u0 + g) * PT, :]
                        nc.sync.dma_start(
                            dst.rearrange("(c q) w -> q c w", q=PT),
                            ot[:, 0:g, :])


def _emit_v5(nc, x_d, bands_d, out_d, planes, h, w, reps=1, deep=False):
    """3 DVE W-adds (6-tap s3); the 7th box tap is folded into the PE
    accumulation as a second shifted-AP matmul chain (12 matmuls/tile).
    deep=True: deeper tile pools so plane/iteration boundaries overlap."""
    nt = h // PT
    assert h % PT == 0 and nt >= 2 and w % CHUNK == 0
    nchunks = w // CHUNK

    AF = mybir.ActivationFunctionType
    OP = mybir.AluOpType

    xb, tb, wb, sb, ab, ob, pb = ((6, 7, 5, 7, 5, 4, 4) if deep
                                  else (4, 5, 3, 5, 3, 3, 3))
    with tile.TileContext(nc) as tc:
        with (
            tc.tile_pool(name="consts", bufs=1) as cp,
            tc.tile_pool(name="xin", bufs=xb) as xp,
            tc.tile_pool(name="thr", bufs=tb) as thp,
            tc.tile_pool(name="wsum", bufs=wb) as wp,
            tc.tile_pool(name="s3p", bufs=sb) as s3p,
            tc.tile_pool(name="absp", bufs=ab) as ap_pool,
            tc.tile_pool(name="outp", bufs=ob) as op_pool,
            tc.tile_pool(name="psum", bufs=pb, space="PSUM") as psp,
        ):
            bands_sb = cp.tile([PT, 5, PT], BF16)
            nc.sync.dma_start(bands_sb[:], bands_d.rearrange("m i j -> i m j"))
            bias_thr = cp.tile([PT, 1], F32)
            nc.gpsimd.memset(bias_thr[:], SIGN_BIAS)
            bias_25 = cp.tile([PT, 1], F32)
            nc.gpsimd.memset(bias_25[:], 25.0)

            rep_ctx = (tc.For_i(0, reps) if reps > 1
                       else contextlib.nullcontext())
            with rep_ctx:
              for p in range(planes):
                x_ring: dict[int, bass.AP] = {}
                ce_ring: dict[int, bass.AP] = {}
                s3_ring: dict[int, bass.AP] = {}
                ot_group: dict[int, bass.AP] = {}
                for t in range(nt + 1):
                    if t < nt:
                        if t % 2 == 0:
                            gsz = 2 if t + 1 < nt else 1
                            xt = xp.tile([PT, 2, w], F32, tag="x")
                            src = x_d[p, t * PT:(t + gsz) * PT, :]
                            nc.sync.dma_start(
                                xt[:, 0:gsz, :],
                                src.rearrange("(c q) w -> q c w", q=PT))
                            x_ring[t] = xt[:, 0, :]
                            if gsz == 2:
                                x_ring[t + 1] = xt[:, 1, :]
                        xv = x_ring[t]

                        # ce holds signs with reflect pad (3 each side)
                        ce = thp.tile([PT, w + 6], BF16, tag="ce")
                        nc.scalar.activation(ce[:, 3:w + 3], xv, AF.Sign,
                                             bias=bias_thr[:])
                        nc.scalar.activation(ce[:, 0:3], ce[:, 6:3:-1],
                                             AF.Copy, bias=0.0)
                        nc.scalar.activation(ce[:, w + 3:w + 6],
                                             ce[:, w + 1:w - 2:-1],
                                             AF.Copy, bias=0.0)
                        ce_ring[t] = ce

                        # 6-tap W-sum s3[c] = sum ce[c..c+5] (3 bf16 adds)
                        s1 = wp.tile([PT, w + 4], BF16, tag="s1")
                        nc.vector.tensor_tensor(s1[:], ce[:, 0:w + 4],
                                                ce[:, 1:w + 5], OP.add)
                        s2 = wp.tile([PT, w], BF16, tag="s2")
                        nc.vector.tensor_tensor(s2[:], s1[:, 0:w],
                                                s1[:, 2:w + 2], OP.add)
                        s3 = s3p.tile([PT, w], BF16, tag="s3")
                        nc.vector.tensor_tensor(s3[:], s2[:], s1[:, 4:w + 4],
                                                OP.add)
                        s3_ring[t] = s3

                    u = t - 1
                    if u < 0:
                        continue
                    if u == 0:
                        mms = [(BT, 0), (BN, 1)]
                    elif u == nt - 1:
                        mms = [(BP, u - 1), (BB, u)]
                    else:
                        mms = [(BP, u - 1), (BM, u), (BN, u + 1)]

                    ps = psp.tile([PT, nchunks, CHUNK], F32, tag="ps")
                    # per-neighbor chains (s3 then raw signs), ordered so the
                    # freshest dependency (tile u+1) issues LAST: the first
                    # chains never wait on s3[u+1]/ce[u+1]
                    chains = []
                    for mi, src_t in mms:          # mms order: u-1, u, u+1
                        chains.append((mi, s3_ring[src_t], 0))
                        chains.append((mi, ce_ring[src_t], 6))
                    for k, (mi, sap, off) in enumerate(chains):
                        for c in range(nchunks):
                            nc.tensor.matmul(
                                ps[:, c, :], bands_sb[:, mi, :],
                                sap[:, c * CHUNK + off:c * CHUNK + off + CHUNK],
                                start=(k == 0), stop=(k == len(chains) - 1))
                    a = ap_pool.tile([PT, w], BF16, tag="a")
                    nc.scalar.activation(a[:], ps.rearrange("q c k -> q (c k)"),
                                         AF.Abs, bias=bias_25[:])

                    if u % 2 == 0:
                        ot = op_pool.tile([PT, 2, w], F32, tag="ot")
                        ot_group[u] = ot
                    else:
                        ot = ot_group[u - 1]
                    nc.vector.scalar_tensor_tensor(ot[:, u % 2, :], a[:], 15.0,
                                                   x_ring[u], OP.is_gt, OP.mult)
                    if u % 2 == 1 or u == nt - 1:
                        u0 = u - (u % 2)
                        g = u - u0 + 1
                        dst = out_d[p, u0 * PT:(u0 + g) * PT, :]
                        nc.sync.dma_start(
                            dst.rearrange("(c q) w -> q c w", q=PT),
                            ot[:, 0:g, :])


def _emit_v14(nc, x_d, bands_d, out_d, planes, h, w, reps=1):
    """v5 with tile-PAIR processing on ACT/DVE: Sign, pads, the 3 W-adds and
    the blend each cover two row-tiles per instruction ([128, 2, w] APs),
    halving DVE/ACT instruction counts and semaphore traffic. PE chains
    unchanged (12 matmuls/tile)."""
    nt = h // PT
    assert h % PT == 0 and nt >= 2 and nt % 2 == 0 and w % CHUNK == 0
    nchunks = w // CHUNK

    AF = mybir.ActivationFunctionType
    OP = mybir.AluOpType

    with tile.TileContext(nc) as tc:
        with (
            tc.tile_pool(name="consts", bufs=1) as cp,
            tc.tile_pool(name="xin", bufs=4) as xp,
            tc.tile_pool(name="thr", bufs=3) as thp,
            tc.tile_pool(name="wsum", bufs=2) as wp,
            tc.tile_pool(name="s3p", bufs=3) as s3p,
            tc.tile_pool(name="absp", bufs=3) as ap_pool,
            tc.tile_pool(name="outp", bufs=3) as op_pool,
            tc.tile_pool(name="psum", bufs=3, space="PSUM") as psp,
        ):
            bands_sb = cp.tile([PT, 5, PT], BF16)
            nc.sync.dma_start(bands_sb[:], bands_d.rearrange("m i j -> i m j"))
            bias_thr = cp.tile([PT, 1], F32)
            nc.gpsimd.memset(bias_thr[:], SIGN_BIAS_SC)
            bias_25 = cp.tile([PT, 1], F32)
            nc.gpsimd.memset(bias_25[:], 25.0)

            rep_ctx = (tc.For_i(0, reps) if reps > 1
                       else contextlib.nullcontext())
            with rep_ctx:
              for p in range(planes):
                x_ring: dict[int, bass.AP] = {}
                ce_ring: dict[int, bass.AP] = {}
                s3_ring: dict[int, bass.AP] = {}
                a_group: dict[int, bass.AP] = {}
                ot_group: dict[int, bass.AP] = {}
                for t in range(nt + 1):
                    if t < nt:
                        if t % 2 == 0:
                            xt = xp.tile([PT, 2, w], F32, tag="x")
                            src = x_d[p, t * PT:(t + 2) * PT, :]
                            nc.sync.dma_start(
                                xt[:], src.rearrange("(c q) w -> q c w", q=PT))
                            x_ring[t] = xt[:, 0, :]
                            x_ring[t + 1] = xt[:, 1, :]
                            x_ring[(t, "pair")] = xt

                            # signs for BOTH tiles in one op + one pad pair
                            ce2 = thp.tile([PT, 2, w + 6], BF16, tag="ce")
                            nc.scalar.activation(ce2[:, :, 3:w + 3], xt[:],
                                                 AF.Sign, bias=bias_thr[:],
                                                 scale=SIGN_SCALE)
                            nc.scalar.activation(ce2[:, :, 0:3],
                                                 ce2[:, :, 6:3:-1],
                                                 AF.Copy, bias=0.0)
                            nc.scalar.activation(ce2[:, :, w + 3:w + 6],
                                                 ce2[:, :, w + 1:w - 2:-1],
                                                 AF.Copy, bias=0.0)
                            ce_ring[t] = ce2[:, 0, :]
                            ce_ring[t + 1] = ce2[:, 1, :]

                            # 6-tap W-sum for both tiles (3 wide bf16 adds)
                            s1 = wp.tile([PT, 2, w + 4], BF16, tag="s1")
                            nc.vector.tensor_tensor(s1[:], ce2[:, :, 0:w + 4],
                                                    ce2[:, :, 1:w + 5],
                                                    OP.add)
                            s2 = wp.tile([PT, 2, w], BF16, tag="s2")
                            nc.vector.tensor_tensor(s2[:], s1[:, :, 0:w],
                                                    s1[:, :, 2:w + 2], OP.add)
                            s3 = s3p.tile([PT, 2, w], BF16, tag="s3")
                            nc.vector.tensor_tensor(s3[:], s2[:],
                                                    s1[:, :, 4:w + 4], OP.add)
                            s3_ring[t] = s3[:, 0, :]
                            s3_ring[t + 1] = s3[:, 1, :]

                    u = t - 1
                    if u < 0:
                        continue
                    if u == 0:
                        mms = [(BT, 0), (BN, 1)]
                    elif u == nt - 1:
                        mms = [(BP, u - 1), (BB, u)]
                    else:
                        mms = [(BP, u - 1), (BM, u), (BN, u + 1)]

                    ps = psp.tile([PT, nchunks, CHUNK], F32, tag="ps")
                    chains = []
                    for mi, src_t in mms:
                        chains.append((mi, s3_ring[src_t], 0))
                        chains.append((mi, ce_ring[src_t], 6))
                    for k, (mi, sap, off) in enumerate(chains):
                        for c in range(nchunks):
                            nc.tensor.matmul(
                                ps[:, c, :], bands_sb[:, mi, :],
                                sap[:, c * CHUNK + off:c * CHUNK + off + CHUNK],
                                start=(k == 0), stop=(k == len(chains) - 1))

                    if u % 2 == 0:
                        a2 = ap_pool.tile([PT, 2, w], BF16, tag="a")
                        a_group[u] = a2
                    else:
                        a2 = a_group[u - 1]
                    nc.scalar.activation(a2[:, u % 2, :],
                                         ps.rearrange("q c k -> q (c k)"),
                                         AF.Abs, bias=bias_25[:])

                    if u % 2 == 1:
                        # one wide blend + one DMA for the whole pair
                        ot = op_pool.tile([PT, 2, w], F32, tag="ot")
                        nc.vector.scalar_tensor_tensor(
                            ot[:], a2[:], 15.0, x_ring[(u - 1, "pair")],
                            OP.is_gt, OP.mult)
                        dst = out_d[p, (u - 1) * PT:(u + 1) * PT, :]
                        nc.sync.dma_start(
                            dst.rearrange("(c q) w -> q c w", q=PT), ot[:])


def _band_pairs(h: int) -> np.ndarray:
    """fp8 band-matrix PAIRS for DoubleRow chains, [9, 2, 128, 128]:
    PR_PM=(BP,BM), PR_ZN=(Z,BN), PR_TN=(BT,BN), PR_PB=(BP,BB), then
    doubled singles (X,X) for same-slot shift pairs."""
    b = _band_blocks(h).astype(np.float32)
    z = np.zeros_like(b[0])
    pairs = np.stack([
        np.stack([b[BP], b[BM]]),
        np.stack([z, b[BN]]),
        np.stack([b[BT], b[BN]]),
        np.stack([b[BP], b[BB]]),
        np.stack([b[BP], b[BP]]),
        np.stack([b[BM], b[BM]]),
        np.stack([b[BN], b[BN]]),
        np.stack([b[BT], b[BT]]),
        np.stack([b[BB], b[BB]]),
    ])
    return pairs.astype(ml_dtypes.float8_e4m3)


def _slot_pair(a: int, ring: int):
    """Slice selecting ring slots (a, (a+1)%ring) in that order."""
    if a < ring - 1:
        return slice(a, a + 2)
    return slice(ring - 1, None, -(ring - 1))


def _emit_v9(nc, x_d, bands_d, bands8_d, out_d, planes, h, w, reps=1,
             alpha_cols=128, taps_dve=3, thin=frozenset()):
    """DVE: 3 bf16 adds (6-tap s3) + blend stt on cols [0, alpha_cols).
    PE: 3 bf16 H-band chains on s3 + 2 fp8 DoubleRow chains covering the
    7th W-tap (raw signs from a second fp8 ACT Sign, slot ring).
    ACT: Sign-bf16, Sign-fp8, pads, Abs(S+25)->a, Sigmoid(45(a-15))->m01
    on cols [alpha_cols, w).
    Pool (GPSIMD): blend out = x*m01 on cols [alpha_cols, w)."""
    nt = h // PT
    assert h % PT == 0 and nt >= 2 and w % CHUNK == 0
    nchunks = w // CHUNK
    RING = 5
    W8 = w + 8

    AF = mybir.ActivationFunctionType
    OP = mybir.AluOpType

    with tile.TileContext(nc) as tc:
        with (
            tc.tile_pool(name="consts", bufs=1) as cp,
            tc.tile_pool(name="xin", bufs=4) as xp,
            tc.tile_pool(name="thr", bufs=3) as thp,
            tc.tile_pool(name="wsum", bufs=3) as wp,
            tc.tile_pool(name="s3p", bufs=5) as s3p,
            tc.tile_pool(name="s7p", bufs=5) as s7p,
            tc.tile_pool(name="absp", bufs=3) as ap_pool,
            tc.tile_pool(name="mp", bufs=3) as mp_pool,
            tc.tile_pool(name="outp", bufs=3) as op_pool,
            tc.tile_pool(name="psum", bufs=3, space="PSUM") as psp,
        ):
            bands_sb = cp.tile([PT, 5, PT], BF16)
            nc.sync.dma_start(bands_sb[:], bands_d.rearrange("m i j -> i m j"))
            if taps_dve != 4:
                bands8_sb = cp.tile([PT, 9, 2, PT], F8)
                nc.sync.dma_start(bands8_sb[:],
                                  bands8_d.rearrange("p m i j -> i p m j"))
                ring8 = cp.tile([PT, RING, W8], F8)
            bias_thr = cp.tile([PT, 1], F32)
            nc.gpsimd.memset(bias_thr[:], SIGN_BIAS_SC)
            bias_25 = cp.tile([PT, 1], F32)
            nc.gpsimd.memset(bias_25[:], 25.0)
            bias_sig = cp.tile([PT, 1], F32)
            nc.gpsimd.memset(bias_sig[:], -3000.0)

            rep_ctx = (tc.For_i(0, reps) if reps > 1
                       else contextlib.nullcontext())
            with rep_ctx:
              for p in range(planes):
                  x_ring: dict[int, bass.AP] = {}
                  s3_ring: dict[int, bass.AP] = {}
                  ot_group: dict[int, bass.AP] = {}
                  for t in range(nt + 1):
                      if t < nt:
                          if t % 2 == 0:
                              gsz = 2 if t + 1 < nt else 1
                              xt = xp.tile([PT, 2, w], F32, tag="x")
                              src = x_d[p, t * PT:(t + gsz) * PT, :]
                              nc.sync.dma_start(
                                  xt[:, 0:gsz, :],
                                  src.rearrange("(c q) w -> q c w", q=PT))
                              x_ring[t] = xt[:, 0, :]
                              if gsz == 2:
                                  x_ring[t + 1] = xt[:, 1, :]
                          xv = x_ring[t]

                          if taps_dve != 4:
                              # fp8 signs into ring slot (cols 3..w+2 + R pad)
                              st = t % RING
                              s8w = 8 if "sign8" in thin else w
                              nc.scalar.activation(ring8[:, st, 3:s8w + 3],
                                                   xv[:, 0:s8w],
                                                   AF.Sign, bias=bias_thr[:],
                                                   scale=SIGN_SCALE)
                              nc.scalar.activation(ring8[:, st, w + 3:w + 6],
                                                   ring8[:, st, w + 1:w - 2:-1],
                                                   AF.Copy, bias=0.0)
                              if taps_dve == 0:
                                  nc.scalar.activation(ring8[:, st, 0:3],
                                                       ring8[:, st, 6:3:-1],
                                                       AF.Copy, bias=0.0)

                          if taps_dve != 0:
                              # bf16 signs with reflect pads
                              ce = thp.tile([PT, w + 6], BF16, tag="ce")
                              sw = 8 if "sign16" in thin else w
                              nc.scalar.activation(ce[:, 3:sw + 3],
                                                   xv[:, 0:sw],
                                                   AF.Sign, bias=bias_thr[:],
                                                   scale=SIGN_SCALE)
                              nc.scalar.activation(ce[:, 0:3], ce[:, 6:3:-1],
                                                   AF.Copy, bias=0.0)
                              nc.scalar.activation(ce[:, w + 3:w + 6],
                                                   ce[:, w + 1:w - 2:-1],
                                                   AF.Copy, bias=0.0)

                              # 6-tap W-sum s3 (3 bf16 adds at 2x)
                              aw = 16 if "adds" in thin else w
                              s1 = wp.tile([PT, w + 4], BF16, tag="s1")
                              nc.vector.tensor_tensor(s1[:, 0:aw + 4],
                                                      ce[:, 0:aw + 4],
                                                      ce[:, 1:aw + 5], OP.add)
                              s2 = wp.tile([PT, w], BF16, tag="s2")
                              nc.vector.tensor_tensor(s2[:, 0:aw],
                                                      s1[:, 0:aw],
                                                      s1[:, 2:aw + 2], OP.add)
                              if taps_dve == 2:
                                  s3_ring[t] = s2
                              else:
                                  s3 = s3p.tile([PT, w], BF16, tag="s3")
                                  nc.vector.tensor_tensor(s3[:, 0:aw],
                                                          s2[:, 0:aw],
                                                          s1[:, 4:aw + 4],
                                                          OP.add)
                              if taps_dve == 4:
                                  s7 = s3p.tile([PT, w], BF16, tag="s7")
                                  nc.vector.tensor_tensor(s7[:, 0:aw],
                                                          s3[:, 0:aw],
                                                          ce[:, 6:aw + 6],
                                                          OP.add)
                                  s3 = s7
                              if taps_dve != 2:
                                  s3_ring[t] = s3

                      u = t - 1
                      if u < 0:
                          continue
                      # chains: kind 'b'=bf16 band, 'd'=fp8 DR cross-slot,
                      # 's'=fp8 DR same-slot shift pair (idx, slot_t, shift)
                      if taps_dve == 0:
                          if u == 0:
                              chains = ([("s", PR_TT, 0, o) for o in (0, 2, 4)]
                                        + [("s", PR_NN, 1, o)
                                           for o in (0, 2, 4)]
                                        + [("d", PR_TN, 0)])
                          elif u == nt - 1:
                              chains = ([("s", PR_PP, u - 1, o)
                                         for o in (0, 2, 4)]
                                        + [("s", PR_B2, u, o)
                                           for o in (0, 2, 4)]
                                        + [("d", PR_PB, u - 1)])
                          else:
                              chains = ([("s", PR_PP, u - 1, o)
                                         for o in (0, 2, 4)]
                                        + [("s", PR_MM, u, o)
                                           for o in (0, 2, 4)]
                                        + [("d", PR_PM, u - 1)]
                                        + [("s", PR_NN, u + 1, o)
                                           for o in (0, 2, 4)]
                                        + [("d", PR_ZN, u)])
                      elif taps_dve == 4:
                          if u == 0:
                              chains = [("b", BT, 0), ("b", BN, 1)]
                          elif u == nt - 1:
                              chains = [("b", BP, u - 1), ("b", BB, u)]
                          else:
                              chains = [("b", BP, u - 1), ("b", BM, u),
                                        ("b", BN, u + 1)]
                      elif taps_dve == 2:
                          if u == 0:
                              chains = ([("d", PR_TN, 0, o) for o in (4, 5, 6)]
                                        + [("b", BT, 0), ("b", BN, 1)])
                          elif u == nt - 1:
                              chains = ([("d", PR_PB, u - 1, o)
                                         for o in (4, 5, 6)]
                                        + [("b", BP, u - 1), ("b", BB, u)])
                          else:
                              chains = ([("d", PR_PM, u - 1, o)
                                         for o in (4, 5, 6)]
                                        + [("d", PR_ZN, u, o)
                                           for o in (4, 5, 6)]
                                        + [("b", BP, u - 1), ("b", BM, u),
                                           ("b", BN, u + 1)])
                      elif u == 0:
                          chains = [("d", PR_TN, 0, 6), ("b", BT, 0),
                                    ("b", BN, 1)]
                      elif u == nt - 1:
                          chains = [("d", PR_PB, u - 1, 6), ("b", BP, u - 1),
                                    ("b", BB, u)]
                      else:
                          chains = [("d", PR_PM, u - 1, 6),
                                    ("d", PR_ZN, u, 6),
                                    ("b", BP, u - 1), ("b", BM, u),
                                    ("b", BN, u + 1)]

                      ps = psp.tile([PT, nchunks, CHUNK], F32, tag="ps")
                      nk = len(chains)
                      for k, ch in enumerate(chains):
                          kind, idx, src_t = ch[0], ch[1], ch[2]
                          for c in range(nchunks):
                              c0 = c * CHUNK
                              if kind == "b":
                                  nc.tensor.matmul(
                                      ps[:, c, :], bands_sb[:, idx, :],
                                      s3_ring[src_t][:, c0:c0 + CHUNK],
                                      start=(k == 0), stop=(k == nk - 1))
                              elif kind == "s":
                                  base = ring8[:, src_t % RING,
                                               c0 + ch[3]:c0 + ch[3] + CHUNK]
                                  rhs = bass.AP(
                                      tensor=base.tensor, offset=base.offset,
                                      ap=[list(base.ap[0]), [1, 2],
                                          [1, CHUNK]])
                                  nc.tensor.matmul(
                                      ps[:, c, :], bands8_sb[:, idx, :, :],
                                      rhs,
                                      start=(k == 0), stop=(k == nk - 1),
                                      perf_mode=DRMODE)
                              else:
                                  sl = _slot_pair(src_t % RING, RING)
                                  sh = ch[3]
                                  nc.tensor.matmul(
                                      ps[:, c, :], bands8_sb[:, idx, :, :],
                                      ring8[:, sl, sh + c0:sh + c0 + CHUNK],
                                      start=(k == 0), stop=(k == nk - 1),
                                      perf_mode=DRMODE)

                      ac = alpha_cols
                      a = ap_pool.tile([PT, w], BF16, tag="a")
                      nc.scalar.activation(a[:], ps.rearrange("q c k -> q (c k)"),
                                           AF.Abs, bias=bias_25[:])
                      m = mp_pool.tile([PT, w], BF16, tag="m")
                      if ac < w:
                          # m01 = sigmoid(200*(a-15)), exactly {0.0, 1.0}:
                          # a is an even integer, so the input is <= -200 or
                          # >= +200; e^-200 underflows f32 to exactly 0.
                          nc.scalar.activation(m[:, ac:w], a[:, ac:w],
                                               AF.Sigmoid, bias=bias_sig[:],
                                               scale=200.0)

                      if u % 2 == 0:
                          ot = op_pool.tile([PT, 2, w], F32, tag="ot")
                          ot_group[u] = ot
                      else:
                          ot = ot_group[u - 1]
                      if ac > 0:
                          nc.vector.scalar_tensor_tensor(
                              ot[:, u % 2, 0:ac], a[:, 0:ac], 15.0,
                              x_ring[u][:, 0:ac], OP.is_gt, OP.mult)
                      if ac < w:
                          nc.gpsimd.tensor_tensor(ot[:, u % 2, ac:w],
                                                  x_ring[u][:, ac:w],
                                                  m[:, ac:w], OP.mult)
                      if u % 2 == 1 or u == nt - 1:
                          u0 = u - (u % 2)
                          g = u - u0 + 1
                          dst = out_d[p, u0 * PT:(u0 + g) * PT, :]
                          nc.sync.dma_start(
                              dst.rearrange("(c q) w -> q c w", q=PT),
                              ot[:, 0:g, :])


def build_module(planes: int = PLANES, h: int = H, w: int = W,
                 version: int = 4) -> bass.Bass:
    """Standalone module for run_bass_kernel_spmd."""
    nc = Bacc()
    x_d = nc.dram_tensor("x", [planes, h, w], F32, kind="ExternalInput")
    bands_d = nc.dram_tensor("bands", [5, PT, PT], BF16, kind="ExternalInput")
    out_d = nc.dram_tensor("out", [planes, h, w], F32, kind="ExternalOutput")
    if version == 9:
        bands8_d = nc.dram_tensor("bands8", [9, 2, PT, PT], F8,
                                  kind="ExternalInput")
        _emit_v9(nc, x_d, bands_d, bands8_d, out_d, planes, h, w)
    else:
        emit = {1: _emit, 2: _emit_v2, 3: _emit_v6, 4: _emit_v5, 5: _emit_v14}[version]
        emit(nc, x_d, bands_d, out_d, planes, h, w)
    nc.finalize()
    return nc


VERSION = int(os.environ.get("BASS_KERNEL_VERSION", "4"))

_MODULES: dict[int, bass.Bass] = {}


def _get_module(version: int | None = None) -> bass.Bass:
    v = VERSION if version is None else version
    if v not in _MODULES:
        _MODULES[v] = build_module(version=v)
    return _MODULES[v]


def _shard_inputs(x: np.ndarray,
                  with_f8: bool = False) -> list[dict[str, np.ndarray]]:
    bands = np.ascontiguousarray(_band_blocks(H))
    bands8 = np.ascontiguousarray(_band_pairs(H)) if with_f8 else None
    in_maps = []
    for i in range(N_CORES):
        shard = np.ascontiguousarray(
            x[i * IMGS_PER_CORE:(i + 1) * IMGS_PER_CORE].reshape(PLANES, H, W))
        m = {"x": shard, "bands": bands}
        if with_f8:
            m["bands8"] = bands8
        in_maps.append(m)
    return in_maps


def run_sharded(x: np.ndarray, version: int | None = None, **spmd_kwargs):
    """Compile+run on cores 0..7; returns (full_output, BassKernelResults)."""
    v = VERSION if version is None else version
    nc = _get_module(v)
    res = run_bass_kernel_spmd(nc, _shard_inputs(x, with_f8=(v == 9)),
                               core_ids=list(range(N_CORES)), **spmd_kwargs)
    out = np.empty((B, C, H, W), np.float32)
    for i in range(N_CORES):
        out[i * IMGS_PER_CORE:(i + 1) * IMGS_PER_CORE] = (
            np.asarray(res.results[i]["out"]).reshape(IMGS_PER_CORE, C, H, W))
    return out, res


def kernel(x) -> np.ndarray:
    x = np.asarray(x, dtype=np.float32)
    assert x.shape == (B, C, H, W), x.shape
    out, _ = run_sharded(x)
    return out


# ---------------------------------------------------------------------------
# Timing harness: chained on-device execution via bass_jit + shard_map.
# (The axon client in this container has no NTFF hook, so HW kernel time is
# measured as per-iteration wall time of a long on-device dependency chain.)
# ---------------------------------------------------------------------------

def measure_kernel_ns(x: np.ndarray, rhi: int = 96, rounds: int = 8,
                      n_per: int = 1, version: int | None = None,
                      emit_kwargs: dict | None = None) -> float:
    """Median on-device kernel time.

    The axon tunnel has a ~5ms per-dispatch floor that hides device time, so
    the kernel pass is repeated `rhi` times inside the NEFF via a For_i
    HARDWARE loop (no instruction growth, pure device re-execution), and
    differenced against a reps=1 NEFF to cancel the dispatch floor:
      kernel_ns = (wall(rhi) - wall(1)) / (rhi - 1).
    """
    import time
    import jax
    import jax.numpy as jnp
    from jax.sharding import Mesh, PartitionSpec
    from concourse import bass2jax

    v = VERSION if version is None else version
    ekw = emit_kwargs or {}
    devices = jax.devices()[:N_CORES]
    mesh = Mesh(np.asarray(devices), ("core",))
    P = PartitionSpec

    def make(reps):
        if v == 9:
            @bass2jax.bass_jit
            def _k(nc, xin, bandsin, bands8in):
                out_d = nc.dram_tensor("out", [PLANES, H, W], F32,
                                       kind="ExternalOutput")
                _emit_v9(nc, xin, bandsin, bands8in, out_d, PLANES, H, W,
                         reps=reps, **ekw)
                return out_d
            nin = 3
        else:
            emit = {1: _emit, 2: _emit_v2, 3: _emit_v6, 4: _emit_v5,
                    5: _emit_v14}[v]

            @bass2jax.bass_jit
            def _k(nc, xin, bandsin):
                out_d = nc.dram_tensor("out", [PLANES, H, W], F32,
                                       kind="ExternalOutput")
                emit(nc, xin, bandsin, out_d, PLANES, H, W, reps=reps, **ekw)
                return out_d
            nin = 2
        return bass2jax.bass_shard_map(_k, mesh=mesh,
                                       in_specs=(P("core"),) * nin,
                                       out_specs=P("core"))

    xg = jnp.asarray(
        np.concatenate([m["x"] for m in _shard_inputs(x)], axis=0))
    bg = jnp.asarray(np.concatenate([_band_blocks(H)] * N_CORES, axis=0))
    args = [bg]
    if v == 9:
        args.append(jnp.asarray(
            np.concatenate([_band_pairs(H)] * N_CORES, axis=0)))
    f1, fh = make(1), make(rhi)
    for f in (f1, fh):
        y = f(xg, *args)
        y.block_until_ready()
        y = f(xg, *args)
        y.block_until_ready()

    def timed(f):
        t0 = time.perf_counter()
        y = f(xg, *args)
        y.block_until_ready()
        return time.perf_counter() - t0

    t1s, ths = [], []
    for _ in range(rounds):
        t1s.append(timed(f1))
        ths.append(timed(fh))
    # Contention noise is strictly additive, so the per-leg minimum over
    # rounds approximates the quiet-machine time for that NEFF.
    m1 = float(np.min(np.array(t1s)))
    mh = float(np.min(np.array(ths)))
    return (mh - m1) / (rhi - 1) * 1e9


def bench_chain(x: np.ndarray, iters: int = 32, warmup: int = 4,
                reps: int = 1):
    import time
    import jax
    import jax.numpy as jnp
    from jax.sharding import Mesh, PartitionSpec
    from concourse import bass2jax

    @bass2jax.bass_jit
    def _jit_kernel(nc, xin, bandsin):
        out_d = nc.dram_tensor("out", [PLANES, H, W], F32,
                               kind="ExternalOutput")
        _emit(nc, xin, bandsin, out_d, PLANES, H, W, reps=reps)
        return out_d

    devices = jax.devices()[:N_CORES]
    mesh = Mesh(np.asarray(devices), ("core",))
    P = PartitionSpec
    f = bass2jax.bass_shard_map(_jit_kernel, mesh=mesh,
                                in_specs=(P("core"), P("core")),
                                out_specs=P("core"))

    xg = jnp.asarray(
        np.concatenate([m["x"] for m in _shard_inputs(x)], axis=0))
    bg = jnp.asarray(np.concatenate([_band_blocks(H)] * N_CORES, axis=0))

    y = f(xg, bg)
    y.block_until_ready()
    for _ in range(warmup):
        y = f(y, bg)
    y.block_until_ready()

    t0 = time.perf_counter()
    for _ in range(iters):
        y = f(y, bg)
    y.block_until_ready()
    dt = (time.perf_counter() - t0) / iters
    return dt, np.asarray(y)

